# revision 21
# baseline (speedup 1.0000x reference)
"""GQA attention kernel for 8 TRN2 NeuronCores.

Sharding (hardcoded): 8 cores = batch(2) x kv-group(4).
Core i handles batch b=i//4, group g=i%4:
  xT  = hidden_states[b].T (bf16, host pre-transposed)   [2048, 2048]
  wqk = permuted [Wq_g | Wk_g | Wv_g] bf16               [2048, 768]
  wo  = row-permuted Wo_g bf16                           [512, 2048]
  rc/rs = RoPE cos/sin tables f32                        [128, 16, 64]
Each core returns a partial output [2048, 2048] f32; host sums the 4
group partials per batch.

Per-core pipeline (matmuls bf16 -> f32 PSUM):
  B) QKV projections in [tok, dim] layout (lhsT = X^T blocks), fused
     RoPE on q+k (6 DVE ops per token tile, host-permuted weight
     columns make the q/k layouts uniform), PE-transpose to Q^T/K^T,
     transpose+V evacuations on the Scalar (ACT) engine which is
     otherwise idle in this phase.
  C) per (q-chunk, kv head): scores^T[k,q] = K^T.T @ Q^T for 2 heads
     into one 2-bank PSUM tile, ONE 1024-wide exp (scale 1/8 fused)
     per head-pair straight out of PSUM, PV via lhsT=V_aug giving
     out^T[d,q] with the softmax row-sum riding in row 64; normalize
     with DVE reciprocal + gpsimd partition-broadcast + DVE multiply.
  D) out_partial = attn^T.T @ Wo interleaved at q-chunk boundaries to
     keep the PE HAM clock-gate warm.
"""

import math
import numpy as np

S = 2048
HID = 2048
NT = 16          # token tiles of 128
NR = 16          # hid tiles of 128
QD = 512         # q dims per core (8 heads x 64)
KD = 128         # kv dims per core (2 kv heads x 64)
D = 64
NQH = 8          # q heads per core

_CACHE = {}


def _build():
    import concourse.bass as bass
    import concourse.mybir as mybir
    from concourse import bacc
    from concourse.tile import TileContext
    from concourse.masks import make_identity

    f32 = mybir.dt.float32
    bf16 = mybir.dt.bfloat16
    AF = mybir.ActivationFunctionType
    OP = mybir.AluOpType

    nc = bacc.Bacc("TRN2", target_bir_lowering=False, debug=False)
    xt = nc.dram_tensor("xt", [HID, S], bf16, kind="ExternalInput").ap()
    wqk = nc.dram_tensor("wqk", [HID, 768], bf16, kind="ExternalInput").ap()
    wo = nc.dram_tensor("wo", [QD, HID], bf16, kind="ExternalInput").ap()
    rc = nc.dram_tensor("rc", [128, NT, D], f32, kind="ExternalInput").ap()
    rsn = nc.dram_tensor("rsn", [128, NT, D], f32, kind="ExternalInput").ap()
    out = nc.dram_tensor("out", [S, HID], f32, kind="ExternalOutput").ap()
    import os
    dbg = os.environ.get("KDEBUG") == "1"
    if dbg:
        d_qt = nc.dram_tensor("d_qt", [128, 4, S], f32, kind="ExternalOutput").ap()
        d_kt = nc.dram_tensor("d_kt", [128, S], f32, kind="ExternalOutput").ap()
        d_v = nc.dram_tensor("d_v", [128, NT, 2, 65], f32, kind="ExternalOutput").ap()
        d_at = nc.dram_tensor("d_at", [128, 4, S], f32, kind="ExternalOutput").ap()

    with TileContext(nc) as tc:
        with (
            tc.tile_pool(name="const", bufs=1) as const,
            tc.tile_pool(name="wts", bufs=1) as wts,
            tc.tile_pool(name="stage", bufs=3) as stage,
            tc.tile_pool(name="tmps", bufs=3) as tmps,
            tc.tile_pool(name="pbf", bufs=4) as pbf,
            tc.tile_pool(name="rbp", bufs=3) as rbp,
            tc.tile_pool(name="outp", bufs=2) as outp,
            tc.tile_pool(name="psS", bufs=2, space="PSUM") as psS,
            tc.tile_pool(name="psO", bufs=4, space="PSUM") as psO,
        ):
            # ---------------- inputs -> SBUF ----------------
            # xT arrives in 4 token-chunks x 16 r-blocks, alternating the
            # two hwdge queues, so phase B's first matmuls start within a
            # few us instead of waiting for the full 8MB.
            wq_sb = wts.tile([128, NR, 768], bf16, tag="wqk")
            nc.scalar.dma_start(
                out=wq_sb[:], in_=wqk.rearrange("(r p) q -> p r q", p=128))
            xT = wts.tile([128, NR, S], bf16, tag="xT")
            for tch in range(4):
                for r in range(NR):
                    # token-chunk 0 entirely on the sync queue so B(t=0)
                    # only waits for wqk (scalar q) and chunk 0 (sync q)
                    eng = nc.sync if (tch == 0 or r % 2 == 0) else nc.scalar
                    eng.dma_start(
                        out=xT[:, r, tch * 512:(tch + 1) * 512],
                        in_=xt[r * 128:(r + 1) * 128,
                               tch * 512:(tch + 1) * 512])
            rc_sb = const.tile([128, NT, D], f32, tag="rc")
            rs_sb = const.tile([128, NT, D], f32, tag="rs")
            nc.gpsimd.dma_start(out=rc_sb[:], in_=rc)
            nc.gpsimd.dma_start(out=rs_sb[:], in_=rsn)
            # wo is not needed until the first D group (~100us in); queue
            # it BEHIND all xT chunks on sync so it doesn't steal HBM
            # bandwidth from the phase-B critical path
            wo_sb = wts.tile([128, 4, HID], bf16, tag="wo")
            nc.sync.dma_start(
                out=wo_sb[:], in_=wo.rearrange("(d p) n -> p d n", p=128))

            ident = const.tile([128, 128], bf16, tag="ident")
            make_identity(nc, ident[:])

            # outputs of phase B
            QT = wts.tile([128, 4, S], bf16, tag="QT")    # [qdim, mt, tok]
            KT = wts.tile([128, S], bf16, tag="KT")       # [kdim(2h), tok]
            V = wts.tile([128, NT, 2, 65], bf16, tag="V")  # [tok128, t, kvh, d+1]
            nc.vector.memset(V[:, :, :, 64:65], 1.0)
            attnT = wts.tile([128, 4, S], bf16, tag="attnT")

            # ---------------- Phase B: QKV + RoPE + transposes --------
            for t in range(NT):
                ps = psS.tile([128, 768], f32, tag="psS")
                for r in range(NR):
                    lt = xT[:, r, t * 128:(t + 1) * 128]
                    nc.tensor.matmul(ps[:, 0:512], lhsT=lt,
                                     rhs=wq_sb[:, r, 0:512],
                                     start=(r == 0), stop=(r == NR - 1))
                    nc.tensor.matmul(ps[:, 512:768], lhsT=lt,
                                     rhs=wq_sb[:, r, 512:768],
                                     start=(r == 0), stop=(r == NR - 1))

                # fused RoPE on q (8 heads) + k (2 heads).
                # ps cols 0:640 are host-permuted to [half=2, blk=5, d=64]
                # (blk 0-3 = q heads, blk 4 = k head of that half).
                # qk staging is [blk=5, half=2, d=64] so that transpose
                # block b holds head-halves ready for QT/KT placement.
                qk = stage.tile([128, 640], bf16, tag="qk")
                v4 = ps[:, 0:640].rearrange(
                    "p (half blk d) -> p half blk d", half=2, d=64)
                o4 = qk[:].rearrange(
                    "p (blk half d) -> p half blk d", half=2, d=64)
                sh = [128, 2, 5, 32]
                ct = rc_sb[:, t, :]
                st = rs_sb[:, t, :]
                c1 = ct[:, None, None, 0:32].broadcast_to(sh)
                s1 = st[:, None, None, 0:32].broadcast_to(sh)
                c2 = ct[:, None, None, 32:64].broadcast_to(sh)
                s2 = st[:, None, None, 32:64].broadcast_to(sh)
                q1, q2 = v4[:, :, :, 0:32], v4[:, :, :, 32:64]
                oa, ob = o4[:, :, :, 0:32], o4[:, :, :, 32:64]
                t1 = tmps.tile(sh, f32, tag="t1")
                t2 = tmps.tile(sh, f32, tag="t2")
                nc.vector.tensor_tensor(t1[:], q1, c1, OP.mult)
                nc.vector.tensor_tensor(t2[:], q2, s1, OP.mult)
                nc.vector.tensor_tensor(oa, t1[:], t2[:], OP.subtract)
                nc.vector.tensor_tensor(t1[:], q2, c2, OP.mult)
                nc.vector.tensor_tensor(t2[:], q1, s2, OP.mult)
                nc.vector.tensor_tensor(ob, t1[:], t2[:], OP.add)
                # V evacuation on ACT (idle in phase B)
                nc.scalar.copy(
                    V[:, t, :, 0:64],
                    ps[:, 640:768].rearrange("p (h d) -> p h d", d=64))
                # transpose q/k blocks into QT/KT; evac on ACT
                tp = psO.tile([128, 640], bf16, tag="acc")
                for db in range(5):
                    nc.tensor.transpose(
                        tp[:, db * 128:(db + 1) * 128],
                        qk[:, db * 128:(db + 1) * 128], ident[:])
                nc.scalar.copy(
                    QT[:, :, t * 128:(t + 1) * 128],
                    tp[:, 0:512].rearrange("p (b j) -> p b j", j=128))
                nc.scalar.copy(KT[:, t * 128:(t + 1) * 128], tp[:, 512:640])

            # ---------------- Phase C/D: attention + Wo ----------------
            o_ts = {}

            def emit_d_group(tt, nch):
                # one quarter of out[tt*128:(tt+1)*128, :] — PE filler
                # injected between attention kt-iterations (its attnT
                # inputs were finalized a block earlier)
                if nch == 0:
                    o_t = outp.tile([128, HID], f32, tag="out")
                    o_ts[tt] = o_t
                o_t = o_ts[tt]
                w_ps = psS.tile([128, 512], f32, tag="psS")
                for db in range(4):
                    nc.tensor.matmul(
                        w_ps[:],
                        lhsT=attnT[:, db, tt * 128:(tt + 1) * 128],
                        rhs=wo_sb[:, db, nch * 512:(nch + 1) * 512],
                        start=(db == 0), stop=(db == 3))
                nc.vector.tensor_copy(
                    o_t[:, nch * 512:(nch + 1) * 512], w_ps[:])
                if nch == 3:
                    nc.sync.dma_start(out=out[tt * 128:(tt + 1) * 128, :],
                                      in_=o_t[:])

            # per (kv, qc) block: 4 q-heads of one kv group over one
            # q-chunk. Emission is software-pipelined: scores(kt) are
            # emitted BEFORE PV(kt-1) so the exp -> scores -> exp critical
            # path never queues behind PV matmuls in the strict-FIFO PE
            # queue. D groups of the previous q-chunk are drip-fed between
            # iterations.
            for qc in range(4):
                for kv in range(2):
                    qr = kv * 64
                    if qc > 0:
                        base = 4 * (qc - 1) + kv * 2
                        dq = [(base + i // 4, i % 4) for i in range(8)]
                    else:
                        dq = []
                    o_ps = []
                    for _i in range(4):
                        acc = psO.tile([65, 512], f32, tag="acc")
                        o_ps.append(acc)
                    plist = [None, None]
                    for kt in range(NT + 1):
                        pnew = [None, None]
                        for pair in range(2):
                            if kt < NT:
                                kblk = KT[kv * 64:(kv + 1) * 64,
                                          kt * 128:(kt + 1) * 128]
                                sp = psS.tile([128, 1024], f32, tag="psS")
                                for j in range(2):
                                    mt = pair * 2 + j
                                    nc.tensor.matmul(
                                        sp[:, j * 512:(j + 1) * 512],
                                        lhsT=kblk,
                                        rhs=QT[qr:qr + 64, mt,
                                               qc * 512:(qc + 1) * 512],
                                        start=True, stop=True)
                                p = pbf.tile([128, 1024], bf16, tag="p")
                                nc.scalar.activation(p[:], sp[:], AF.Exp,
                                                     scale=0.125)
                                pnew[pair] = p
                            if kt > 0:
                                for j in range(2):
                                    nc.tensor.matmul(
                                        o_ps[pair * 2 + j][:],
                                        lhsT=V[:, kt - 1, kv, :],
                                        rhs=plist[pair][:,
                                                        j * 512:(j + 1) * 512],
                                        start=(kt == 1), stop=(kt == NT))
                            # inject a D group in the PE-slack slot between
                            # the two score pairs
                            if (pair == 0 and dq and kt >= 2
                                    and kt % 2 == 0):
                                emit_d_group(*dq.pop(0))
                        plist = pnew
                    for h4 in range(4):
                        rsum = rbp.tile([1, 512], f32, tag="rsum")
                        nc.vector.tensor_copy(rsum[:], o_ps[h4][64:65, :])
                        recip = rbp.tile([1, 512], f32, tag="recip")
                        nc.vector.reciprocal_approx_fast(recip[:], rsum[:])
                        rb = rbp.tile([64, 512], f32, tag="rb")
                        nc.gpsimd.partition_broadcast(rb[:], recip[:])
                        nc.vector.tensor_tensor(
                            attnT[qr:qr + 64, h4, qc * 512:(qc + 1) * 512],
                            o_ps[h4][0:64, :], rb[:], OP.mult)
            for i in range(16):
                emit_d_group(12 + i // 4, i % 4)

            if dbg:
                for (dtile, stile) in ((d_qt, QT), (d_kt, KT), (d_v, V),
                                       (d_at, attnT)):
                    nc.gpsimd.dma_start(out=dtile, in_=stile[:])

    nc.compile()
    return nc


def _get_nc():
    if "nc" not in _CACHE:
        _CACHE["nc"] = _build()
    return _CACHE["nc"]


def _rope_tables():
    # cos/sin[p, t, i] at position t*128+p, emb = concat(freqs, freqs)
    inv = 1.0 / (10000.0 ** (np.arange(0, 32, dtype=np.float64) / 32.0))
    pos = np.arange(S, dtype=np.float64)
    fr = np.outer(pos, inv)                       # [S, 32]
    emb = np.concatenate([fr, fr], axis=1)        # [S, 64]
    cos = np.cos(emb).astype(np.float32).reshape(NT, 128, D).transpose(1, 0, 2)
    sin = np.sin(emb).astype(np.float32).reshape(NT, 128, D).transpose(1, 0, 2)
    return np.ascontiguousarray(cos), np.ascontiguousarray(sin)


def _shard(inputs):
    import ml_dtypes
    bf = ml_dtypes.bfloat16
    hs = np.asarray(inputs["hidden_states"], np.float32)
    Wq = np.asarray(inputs["Wq"], np.float32)
    Wk = np.asarray(inputs["Wk"], np.float32)
    Wv = np.asarray(inputs["Wv"], np.float32)
    Wo = np.asarray(inputs["Wo"], np.float32)
    cos, sin = _rope_tables()
    xts = [np.ascontiguousarray(hs[b].T).astype(bf) for b in range(2)]
    in_maps = []
    for i in range(8):
        b, g = divmod(i, 4)
        # wqk columns: [half=2, blk=5, d=64]; blk 0-3 = q head h=half*4+blk,
        # blk 4 = k head kh=half. then v (2 heads x 64) appended.
        cols = []
        for half in range(2):
            for blk in range(5):
                if blk < 4:
                    h = half * 4 + blk
                    cols.append(Wq[:, g * 512 + h * 64: g * 512 + (h + 1) * 64])
                else:
                    cols.append(Wk[:, g * 128 + half * 64:
                                   g * 128 + (half + 1) * 64])
        cols.append(Wv[:, g * 128:(g + 1) * 128])
        wqk = np.concatenate(cols, axis=1).astype(bf)
        wo = np.ascontiguousarray(
            Wo[g * 512:(g + 1) * 512, :].reshape(8, 64, HID)[
                [0, 4, 1, 5, 2, 6, 3, 7]].reshape(512, HID)).astype(bf)
        in_maps.append({
            "xt": xts[b],
            "wqk": np.ascontiguousarray(wqk),
            "wo": wo,
            "rc": cos,
            "rsn": sin,
        })
    return in_maps


def run(inputs, trace=False, tmpdir=None):
    """Run on 8 cores; returns (output [2,2048,2048] f32, exec_time_ns)."""
    from concourse.bass_utils import run_bass_kernel_spmd

    nc = _get_nc()
    in_maps = _shard(inputs)
    kwargs = {}
    if trace:
        import sys, types
        from trn_agent_boot.trn_boot import _ntff_profile_via_ctypes
        if "antenv.axon_hooks" not in sys.modules:
            mod = types.ModuleType("antenv.axon_hooks")
            hook = _ntff_profile_via_ctypes("/opt/axon/libaxon_pjrt.so")
            mod.get_axon_ntff_profile_hook = lambda: hook
            sys.modules["antenv.axon_hooks"] = mod
        import concourse.bass_utils as bu
        bu.upload_artifacts = lambda d: f"local://{d}"
        kwargs = {"trace": True, "tmpdir": tmpdir}
    res = run_bass_kernel_spmd(nc, in_maps, core_ids=list(range(8)), **kwargs)
    full = np.zeros((2, S, HID), np.float32)
    for i in range(8):
        b = i // 4
        full[b] += res.results[i]["out"]
    return full, res.exec_time_ns


def kernel(**inputs):
    out, _ = run(inputs)
    return out


# revision 22
# speedup vs baseline: 1.1100x; 1.1100x over previous
"""GQA attention kernel for 8 TRN2 NeuronCores.

Sharding (hardcoded): 8 cores = batch(2) x kv-group(4).
Core i handles batch b=i//4, group g=i%4:
  xT  = hidden_states[b].T (bf16, host pre-transposed)   [2048, 2048]
  wqk = permuted [Wq_g | Wk_g | Wv_g] bf16               [2048, 768]
  wo  = row-permuted Wo_g bf16                           [512, 2048]
  rc/rs = RoPE cos/sin tables f32                        [128, 16, 64]
Each core returns a partial output [2048, 2048] f32; host sums the 4
group partials per batch.

Per-core pipeline (matmuls bf16 -> f32 PSUM):
  B) QKV projections in [tok, dim] layout (lhsT = X^T blocks), fused
     RoPE on q+k (6 DVE ops per token tile, host-permuted weight
     columns make the q/k layouts uniform), PE-transpose to Q^T/K^T,
     transpose+V evacuations on the Scalar (ACT) engine which is
     otherwise idle in this phase.
  C) per (q-chunk, kv head): scores^T[k,q] = K^T.T @ Q^T for 2 heads
     into one 2-bank PSUM tile, ONE 1024-wide exp (scale 1/8 fused)
     per head-pair straight out of PSUM, PV via lhsT=V_aug giving
     out^T[d,q] with the softmax row-sum riding in row 64; normalize
     with DVE reciprocal + gpsimd partition-broadcast + DVE multiply.
  D) out_partial = attn^T.T @ Wo interleaved at q-chunk boundaries to
     keep the PE HAM clock-gate warm.
"""

import math
import numpy as np

S = 2048
HID = 2048
NT = 16          # token tiles of 128
NR = 16          # hid tiles of 128
QD = 512         # q dims per core (8 heads x 64)
KD = 128         # kv dims per core (2 kv heads x 64)
D = 64
NQH = 8          # q heads per core

_CACHE = {}


def _build():
    import concourse.bass as bass
    import concourse.mybir as mybir
    from concourse import bacc
    from concourse.tile import TileContext
    from concourse.masks import make_identity

    f32 = mybir.dt.float32
    bf16 = mybir.dt.bfloat16
    AF = mybir.ActivationFunctionType
    OP = mybir.AluOpType

    nc = bacc.Bacc("TRN2", target_bir_lowering=False, debug=False)
    xt = nc.dram_tensor("xt", [HID, S], bf16, kind="ExternalInput").ap()
    wqk = nc.dram_tensor("wqk", [HID, 768], bf16, kind="ExternalInput").ap()
    wo = nc.dram_tensor("wo", [QD, HID], bf16, kind="ExternalInput").ap()
    rc = nc.dram_tensor("rc", [128, NT, D], f32, kind="ExternalInput").ap()
    rsn = nc.dram_tensor("rsn", [128, NT, D], f32, kind="ExternalInput").ap()
    out = nc.dram_tensor("out", [S, HID], f32, kind="ExternalOutput").ap()
    import os
    dbg = os.environ.get("KDEBUG") == "1"
    if dbg:
        d_qt = nc.dram_tensor("d_qt", [128, 4, S], f32, kind="ExternalOutput").ap()
        d_kt = nc.dram_tensor("d_kt", [128, S], f32, kind="ExternalOutput").ap()
        d_v = nc.dram_tensor("d_v", [128, NT, 2, 65], f32, kind="ExternalOutput").ap()
        d_at = nc.dram_tensor("d_at", [128, 4, S], f32, kind="ExternalOutput").ap()

    with TileContext(nc) as tc:
        with (
            tc.tile_pool(name="const", bufs=1) as const,
            tc.tile_pool(name="wts", bufs=1) as wts,
            tc.tile_pool(name="stage", bufs=3) as stage,
            tc.tile_pool(name="tmps", bufs=3) as tmps,
            tc.tile_pool(name="pbf", bufs=4) as pbf,
            tc.tile_pool(name="rbp", bufs=3) as rbp,
            tc.tile_pool(name="outp", bufs=2) as outp,
            tc.tile_pool(name="psS", bufs=2, space="PSUM") as psS,
            tc.tile_pool(name="psO", bufs=4, space="PSUM") as psO,
        ):
            # ---------------- inputs -> SBUF ----------------
            # xT arrives in 4 token-chunks x 16 r-blocks, alternating the
            # two hwdge queues, so phase B's first matmuls start within a
            # few us instead of waiting for the full 8MB.
            wq_sb = wts.tile([128, NR, 768], bf16, tag="wqk")
            nc.scalar.dma_start(
                out=wq_sb[:], in_=wqk.rearrange("(r p) q -> p r q", p=128))
            xT = wts.tile([128, NR, S], bf16, tag="xT")
            for tch in range(4):
                for r in range(NR):
                    # token-chunk 0 entirely on the sync queue so B(t=0)
                    # only waits for wqk (scalar q) and chunk 0 (sync q)
                    eng = nc.sync if (tch == 0 or r % 2 == 0) else nc.scalar
                    eng.dma_start(
                        out=xT[:, r, tch * 512:(tch + 1) * 512],
                        in_=xt[r * 128:(r + 1) * 128,
                               tch * 512:(tch + 1) * 512])
            rc_sb = const.tile([128, NT, D], f32, tag="rc")
            rs_sb = const.tile([128, NT, D], f32, tag="rs")
            nc.gpsimd.dma_start(out=rc_sb[:], in_=rc)
            nc.gpsimd.dma_start(out=rs_sb[:], in_=rsn)
            # wo is not needed until the first D group (~100us in); queue
            # it BEHIND all xT chunks on sync so it doesn't steal HBM
            # bandwidth from the phase-B critical path
            wo_sb = wts.tile([128, 4, HID], bf16, tag="wo")
            nc.sync.dma_start(
                out=wo_sb[:], in_=wo.rearrange("(d p) n -> p d n", p=128))

            ident = const.tile([128, 128], bf16, tag="ident")
            make_identity(nc, ident[:])

            # outputs of phase B
            QT = wts.tile([128, 4, S], bf16, tag="QT")    # [qdim, mt, tok]
            KT = wts.tile([128, S], bf16, tag="KT")       # [kdim(2h), tok]
            V = wts.tile([128, NT, 2, 65], bf16, tag="V")  # [tok128, t, kvh, d+1]
            nc.vector.memset(V[:, :, :, 64:65], 1.0)
            attnT = wts.tile([128, 4, S], bf16, tag="attnT")

            # ---------------- Phase B: QKV + RoPE + transposes --------
            for t in range(NT):
                ps = psS.tile([128, 768], f32, tag="psS")
                for r in range(NR):
                    lt = xT[:, r, t * 128:(t + 1) * 128]
                    nc.tensor.matmul(ps[:, 0:512], lhsT=lt,
                                     rhs=wq_sb[:, r, 0:512],
                                     start=(r == 0), stop=(r == NR - 1))
                    nc.tensor.matmul(ps[:, 512:768], lhsT=lt,
                                     rhs=wq_sb[:, r, 512:768],
                                     start=(r == 0), stop=(r == NR - 1))

                # fused RoPE on q (8 heads) + k (2 heads).
                # ps cols 0:640 are host-permuted to [half=2, blk=5, d=64]
                # (blk 0-3 = q heads, blk 4 = k head of that half).
                # qk staging is [blk=5, half=2, d=64] so that transpose
                # block b holds head-halves ready for QT/KT placement.
                qk = stage.tile([128, 640], bf16, tag="qk")
                v4 = ps[:, 0:640].rearrange(
                    "p (half blk d) -> p half blk d", half=2, d=64)
                o4 = qk[:].rearrange(
                    "p (blk half d) -> p half blk d", half=2, d=64)
                sh = [128, 2, 5, 32]
                ct = rc_sb[:, t, :]
                st = rs_sb[:, t, :]
                c1 = ct[:, None, None, 0:32].broadcast_to(sh)
                s1 = st[:, None, None, 0:32].broadcast_to(sh)
                c2 = ct[:, None, None, 32:64].broadcast_to(sh)
                s2 = st[:, None, None, 32:64].broadcast_to(sh)
                q1, q2 = v4[:, :, :, 0:32], v4[:, :, :, 32:64]
                oa, ob = o4[:, :, :, 0:32], o4[:, :, :, 32:64]
                t1 = tmps.tile(sh, f32, tag="t1")
                t2 = tmps.tile(sh, f32, tag="t2")
                nc.vector.tensor_tensor(t1[:], q1, c1, OP.mult)
                nc.vector.tensor_tensor(t2[:], q2, s1, OP.mult)
                nc.vector.tensor_tensor(oa, t1[:], t2[:], OP.subtract)
                nc.vector.tensor_tensor(t1[:], q2, c2, OP.mult)
                nc.vector.tensor_tensor(t2[:], q1, s2, OP.mult)
                nc.vector.tensor_tensor(ob, t1[:], t2[:], OP.add)
                # V evacuation on ACT (idle in phase B)
                nc.scalar.copy(
                    V[:, t, :, 0:64],
                    ps[:, 640:768].rearrange("p (h d) -> p h d", d=64))
                # transpose q/k blocks into QT/KT; evac on ACT
                tp = psO.tile([128, 640], bf16, tag="acc")
                for db in range(5):
                    nc.tensor.transpose(
                        tp[:, db * 128:(db + 1) * 128],
                        qk[:, db * 128:(db + 1) * 128], ident[:])
                nc.scalar.copy(
                    QT[:, :, t * 128:(t + 1) * 128],
                    tp[:, 0:512].rearrange("p (b j) -> p b j", j=128))
                nc.scalar.copy(KT[:, t * 128:(t + 1) * 128], tp[:, 512:640])

            # ---------------- Phase C/D: attention + Wo ----------------
            o_ts = {}

            def emit_d_group(tt, nch):
                # one quarter of out[tt*128:(tt+1)*128, :] — PE filler
                # injected between attention kt-iterations (its attnT
                # inputs were finalized a block earlier)
                if nch == 0:
                    o_t = outp.tile([128, HID], f32, tag="out")
                    o_ts[tt] = o_t
                o_t = o_ts[tt]
                w_ps = psS.tile([128, 512], f32, tag="psS")
                for db in range(4):
                    nc.tensor.matmul(
                        w_ps[:],
                        lhsT=attnT[:, db, tt * 128:(tt + 1) * 128],
                        rhs=wo_sb[:, db, nch * 512:(nch + 1) * 512],
                        start=(db == 0), stop=(db == 3))
                nc.vector.tensor_copy(
                    o_t[:, nch * 512:(nch + 1) * 512], w_ps[:])
                if nch == 3:
                    nc.sync.dma_start(out=out[tt * 128:(tt + 1) * 128, :],
                                      in_=o_t[:])

            # per (kv, qc) block: 4 q-heads of one kv group over one
            # q-chunk. Emission is software-pipelined: scores(kt) are
            # emitted BEFORE PV(kt-1) so the exp -> scores -> exp critical
            # path never queues behind PV matmuls in the strict-FIFO PE
            # queue. D groups of the previous q-chunk are drip-fed between
            # iterations.
            for qc in range(4):
                for kv in range(2):
                    qr = kv * 64
                    if qc > 0:
                        base = 4 * (qc - 1) + kv * 2
                        dq = [(base + i // 4, i % 4) for i in range(8)]
                    else:
                        dq = []
                    o_ps = []
                    for _i in range(4):
                        acc = psO.tile([65, 512], f32, tag="acc")
                        o_ps.append(acc)
                    plist = [None, None]
                    for kt in range(NT + 1):
                        pnew = [None, None]
                        for pair in range(2):
                            if kt < NT:
                                kblk = KT[kv * 64:(kv + 1) * 64,
                                          kt * 128:(kt + 1) * 128]
                                sp = psS.tile([128, 1024], f32, tag="psS")
                                for j in range(2):
                                    mt = pair * 2 + j
                                    nc.tensor.matmul(
                                        sp[:, j * 512:(j + 1) * 512],
                                        lhsT=kblk,
                                        rhs=QT[qr:qr + 64, mt,
                                               qc * 512:(qc + 1) * 512],
                                        start=True, stop=True)
                                p = pbf.tile([128, 1024], bf16, tag="p")
                                nc.scalar.activation(p[:], sp[:], AF.Exp,
                                                     scale=0.125)
                                pnew[pair] = p
                            if kt > 0:
                                for j in range(2):
                                    nc.tensor.matmul(
                                        o_ps[pair * 2 + j][:],
                                        lhsT=V[:, kt - 1, kv, :],
                                        rhs=plist[pair][:,
                                                        j * 512:(j + 1) * 512],
                                        start=(kt == 1), stop=(kt == NT))
                        plist = pnew
                        if dq and kt >= 2 and kt % 2 == 0:
                            emit_d_group(*dq.pop(0))
                    for h4 in range(4):
                        rsum = rbp.tile([1, 512], f32, tag="rsum")
                        nc.vector.tensor_copy(rsum[:], o_ps[h4][64:65, :])
                        recip = rbp.tile([1, 512], f32, tag="recip")
                        nc.vector.reciprocal_approx_fast(recip[:], rsum[:])
                        rb = rbp.tile([64, 512], f32, tag="rb")
                        nc.gpsimd.partition_broadcast(rb[:], recip[:])
                        nc.vector.tensor_tensor(
                            attnT[qr:qr + 64, h4, qc * 512:(qc + 1) * 512],
                            o_ps[h4][0:64, :], rb[:], OP.mult)
            for i in range(16):
                emit_d_group(12 + i // 4, i % 4)

            if dbg:
                for (dtile, stile) in ((d_qt, QT), (d_kt, KT), (d_v, V),
                                       (d_at, attnT)):
                    nc.gpsimd.dma_start(out=dtile, in_=stile[:])

    nc.compile()
    return nc


def _get_nc():
    if "nc" not in _CACHE:
        _CACHE["nc"] = _build()
    return _CACHE["nc"]


def _rope_tables():
    # cos/sin[p, t, i] at position t*128+p, emb = concat(freqs, freqs)
    inv = 1.0 / (10000.0 ** (np.arange(0, 32, dtype=np.float64) / 32.0))
    pos = np.arange(S, dtype=np.float64)
    fr = np.outer(pos, inv)                       # [S, 32]
    emb = np.concatenate([fr, fr], axis=1)        # [S, 64]
    cos = np.cos(emb).astype(np.float32).reshape(NT, 128, D).transpose(1, 0, 2)
    sin = np.sin(emb).astype(np.float32).reshape(NT, 128, D).transpose(1, 0, 2)
    return np.ascontiguousarray(cos), np.ascontiguousarray(sin)


def _shard(inputs):
    import ml_dtypes
    bf = ml_dtypes.bfloat16
    hs = np.asarray(inputs["hidden_states"], np.float32)
    Wq = np.asarray(inputs["Wq"], np.float32)
    Wk = np.asarray(inputs["Wk"], np.float32)
    Wv = np.asarray(inputs["Wv"], np.float32)
    Wo = np.asarray(inputs["Wo"], np.float32)
    cos, sin = _rope_tables()
    xts = [np.ascontiguousarray(hs[b].T).astype(bf) for b in range(2)]
    in_maps = []
    for i in range(8):
        b, g = divmod(i, 4)
        # wqk columns: [half=2, blk=5, d=64]; blk 0-3 = q head h=half*4+blk,
        # blk 4 = k head kh=half. then v (2 heads x 64) appended.
        cols = []
        for half in range(2):
            for blk in range(5):
                if blk < 4:
                    h = half * 4 + blk
                    cols.append(Wq[:, g * 512 + h * 64: g * 512 + (h + 1) * 64])
                else:
                    cols.append(Wk[:, g * 128 + half * 64:
                                   g * 128 + (half + 1) * 64])
        cols.append(Wv[:, g * 128:(g + 1) * 128])
        wqk = np.concatenate(cols, axis=1).astype(bf)
        wo = np.ascontiguousarray(
            Wo[g * 512:(g + 1) * 512, :].reshape(8, 64, HID)[
                [0, 4, 1, 5, 2, 6, 3, 7]].reshape(512, HID)).astype(bf)
        in_maps.append({
            "xt": xts[b],
            "wqk": np.ascontiguousarray(wqk),
            "wo": wo,
            "rc": cos,
            "rsn": sin,
        })
    return in_maps


def run(inputs, trace=False, tmpdir=None):
    """Run on 8 cores; returns (output [2,2048,2048] f32, exec_time_ns)."""
    from concourse.bass_utils import run_bass_kernel_spmd

    nc = _get_nc()
    in_maps = _shard(inputs)
    kwargs = {}
    if trace:
        import sys, types
        from trn_agent_boot.trn_boot import _ntff_profile_via_ctypes
        if "antenv.axon_hooks" not in sys.modules:
            mod = types.ModuleType("antenv.axon_hooks")
            hook = _ntff_profile_via_ctypes("/opt/axon/libaxon_pjrt.so")
            mod.get_axon_ntff_profile_hook = lambda: hook
            sys.modules["antenv.axon_hooks"] = mod
        import concourse.bass_utils as bu
        bu.upload_artifacts = lambda d: f"local://{d}"
        kwargs = {"trace": True, "tmpdir": tmpdir}
    res = run_bass_kernel_spmd(nc, in_maps, core_ids=list(range(8)), **kwargs)
    full = np.zeros((2, S, HID), np.float32)
    for i in range(8):
        b = i // 4
        full[b] += res.results[i]["out"]
    return full, res.exec_time_ns


def kernel(**inputs):
    out, _ = run(inputs)
    return out


# revision 23
# speedup vs baseline: 1.1447x; 1.0313x over previous
"""GQA attention kernel for 8 TRN2 NeuronCores.

Sharding (hardcoded): 8 cores = batch(2) x kv-group(4).
Core i handles batch b=i//4, group g=i%4:
  xT  = hidden_states[b].T (bf16, host pre-transposed)   [2048, 2048]
  wqk = permuted [Wq_g | Wk_g | Wv_g] bf16               [2048, 768]
  wo  = row-permuted Wo_g bf16                           [512, 2048]
  rc/rs = RoPE cos/sin tables f32                        [128, 16, 64]
Each core returns a partial output [2048, 2048] f32; host sums the 4
group partials per batch.

Per-core pipeline (matmuls bf16 -> f32 PSUM):
  B) QKV projections in [tok, dim] layout (lhsT = X^T blocks), fused
     RoPE on q+k (6 DVE ops per token tile, host-permuted weight
     columns make the q/k layouts uniform), PE-transpose to Q^T/K^T,
     transpose+V evacuations on the Scalar (ACT) engine which is
     otherwise idle in this phase.
  C) per (q-chunk, kv head): scores^T[k,q] = K^T.T @ Q^T for 2 heads
     into one 2-bank PSUM tile, ONE 1024-wide exp (scale 1/8 fused)
     per head-pair straight out of PSUM, PV via lhsT=V_aug giving
     out^T[d,q] with the softmax row-sum riding in row 64; normalize
     with DVE reciprocal + gpsimd partition-broadcast + DVE multiply.
  D) out_partial = attn^T.T @ Wo interleaved at q-chunk boundaries to
     keep the PE HAM clock-gate warm.
"""

import math
import numpy as np

S = 2048
HID = 2048
NT = 16          # token tiles of 128
NR = 16          # hid tiles of 128
QD = 512         # q dims per core (8 heads x 64)
KD = 128         # kv dims per core (2 kv heads x 64)
D = 64
NQH = 8          # q heads per core

_CACHE = {}


def _build():
    import concourse.bass as bass
    import concourse.mybir as mybir
    from concourse import bacc
    from concourse.tile import TileContext
    from concourse.masks import make_identity

    f32 = mybir.dt.float32
    bf16 = mybir.dt.bfloat16
    AF = mybir.ActivationFunctionType
    OP = mybir.AluOpType

    nc = bacc.Bacc("TRN2", target_bir_lowering=False, debug=False)
    xt = nc.dram_tensor("xt", [HID, S], bf16, kind="ExternalInput").ap()
    wqk = nc.dram_tensor("wqk", [HID, 768], bf16, kind="ExternalInput").ap()
    wo = nc.dram_tensor("wo", [QD, HID], bf16, kind="ExternalInput").ap()
    rc = nc.dram_tensor("rc", [128, NT, D], f32, kind="ExternalInput").ap()
    rsn = nc.dram_tensor("rsn", [128, NT, D], f32, kind="ExternalInput").ap()
    out = nc.dram_tensor("out", [S, HID], f32, kind="ExternalOutput").ap()
    import os
    dbg = os.environ.get("KDEBUG") == "1"
    if dbg:
        d_qt = nc.dram_tensor("d_qt", [128, 4, S], f32, kind="ExternalOutput").ap()
        d_kt = nc.dram_tensor("d_kt", [128, S], f32, kind="ExternalOutput").ap()
        d_v = nc.dram_tensor("d_v", [128, NT, 2, 65], f32, kind="ExternalOutput").ap()
        d_at = nc.dram_tensor("d_at", [128, 4, S], f32, kind="ExternalOutput").ap()

    with TileContext(nc) as tc:
        with (
            tc.tile_pool(name="const", bufs=1) as const,
            tc.tile_pool(name="wts", bufs=1) as wts,
            tc.tile_pool(name="stage", bufs=3) as stage,
            tc.tile_pool(name="tmps", bufs=3) as tmps,
            tc.tile_pool(name="pbf", bufs=4) as pbf,
            tc.tile_pool(name="rbp", bufs=3) as rbp,
            tc.tile_pool(name="outp", bufs=2) as outp,
            tc.tile_pool(name="psS", bufs=2, space="PSUM") as psS,
            tc.tile_pool(name="psO", bufs=4, space="PSUM") as psO,
        ):
            # ---------------- inputs -> SBUF ----------------
            # xT arrives in 4 token-chunks x 16 r-blocks, alternating the
            # two hwdge queues, so phase B's first matmuls start within a
            # few us instead of waiting for the full 8MB.
            wq_sb = wts.tile([128, NR, 768], bf16, tag="wqk")
            nc.scalar.dma_start(
                out=wq_sb[:], in_=wqk.rearrange("(r p) q -> p r q", p=128))
            xT = wts.tile([128, NR, S], bf16, tag="xT")
            for tch in range(4):
                for r in range(NR):
                    # token-chunk 0 entirely on the sync queue so B(t=0)
                    # only waits for wqk (scalar q) and chunk 0 (sync q)
                    eng = nc.sync if (tch == 0 or r % 2 == 0) else nc.scalar
                    eng.dma_start(
                        out=xT[:, r, tch * 512:(tch + 1) * 512],
                        in_=xt[r * 128:(r + 1) * 128,
                               tch * 512:(tch + 1) * 512])
            rc_sb = const.tile([128, NT, D], f32, tag="rc")
            rs_sb = const.tile([128, NT, D], f32, tag="rs")
            nc.gpsimd.dma_start(out=rc_sb[:], in_=rc)
            nc.gpsimd.dma_start(out=rs_sb[:], in_=rsn)
            # wo is not needed until the first D group (~100us in)
            wo_sb = wts.tile([128, 4, HID], bf16, tag="wo")
            nc.gpsimd.dma_start(
                out=wo_sb[:], in_=wo.rearrange("(d p) n -> p d n", p=128))

            ident = const.tile([128, 128], bf16, tag="ident")
            make_identity(nc, ident[:])

            # outputs of phase B
            QT = wts.tile([128, 4, S], bf16, tag="QT")    # [qdim, mt, tok]
            KT = wts.tile([128, S], bf16, tag="KT")       # [kdim(2h), tok]
            V = wts.tile([128, NT, 2, 65], bf16, tag="V")  # [tok128, t, kvh, d+1]
            nc.vector.memset(V[:, :, :, 64:65], 1.0)
            attnT = wts.tile([128, 4, S], bf16, tag="attnT")

            # ---------------- Phase B: QKV + RoPE + transposes --------
            for t in range(NT):
                ps = psS.tile([128, 768], f32, tag="psS")
                for r in range(NR):
                    lt = xT[:, r, t * 128:(t + 1) * 128]
                    nc.tensor.matmul(ps[:, 0:512], lhsT=lt,
                                     rhs=wq_sb[:, r, 0:512],
                                     start=(r == 0), stop=(r == NR - 1))
                    nc.tensor.matmul(ps[:, 512:768], lhsT=lt,
                                     rhs=wq_sb[:, r, 512:768],
                                     start=(r == 0), stop=(r == NR - 1))

                # fused RoPE on q (8 heads) + k (2 heads).
                # ps cols 0:640 are host-permuted to [half=2, blk=5, d=64]
                # (blk 0-3 = q heads, blk 4 = k head of that half).
                # qk staging is [blk=5, half=2, d=64] so that transpose
                # block b holds head-halves ready for QT/KT placement.
                qk = stage.tile([128, 640], bf16, tag="qk")
                v4 = ps[:, 0:640].rearrange(
                    "p (half blk d) -> p half blk d", half=2, d=64)
                o4 = qk[:].rearrange(
                    "p (blk half d) -> p half blk d", half=2, d=64)
                sh = [128, 2, 5, 32]
                ct = rc_sb[:, t, :]
                st = rs_sb[:, t, :]
                c1 = ct[:, None, None, 0:32].broadcast_to(sh)
                s1 = st[:, None, None, 0:32].broadcast_to(sh)
                c2 = ct[:, None, None, 32:64].broadcast_to(sh)
                s2 = st[:, None, None, 32:64].broadcast_to(sh)
                q1, q2 = v4[:, :, :, 0:32], v4[:, :, :, 32:64]
                oa, ob = o4[:, :, :, 0:32], o4[:, :, :, 32:64]
                t1 = tmps.tile(sh, f32, tag="t1")
                t2 = tmps.tile(sh, f32, tag="t2")
                nc.vector.tensor_tensor(t1[:], q1, c1, OP.mult)
                nc.vector.tensor_tensor(t2[:], q2, s1, OP.mult)
                nc.vector.tensor_tensor(oa, t1[:], t2[:], OP.subtract)
                nc.vector.tensor_tensor(t1[:], q2, c2, OP.mult)
                nc.vector.tensor_tensor(t2[:], q1, s2, OP.mult)
                nc.vector.tensor_tensor(ob, t1[:], t2[:], OP.add)
                # V evacuation on ACT (idle in phase B)
                nc.scalar.copy(
                    V[:, t, :, 0:64],
                    ps[:, 640:768].rearrange("p (h d) -> p h d", d=64))
                # transpose q/k blocks into QT/KT; evac on ACT
                tp = psO.tile([128, 640], bf16, tag="acc")
                for db in range(5):
                    nc.tensor.transpose(
                        tp[:, db * 128:(db + 1) * 128],
                        qk[:, db * 128:(db + 1) * 128], ident[:])
                nc.scalar.copy(
                    QT[:, :, t * 128:(t + 1) * 128],
                    tp[:, 0:512].rearrange("p (b j) -> p b j", j=128))
                nc.scalar.copy(KT[:, t * 128:(t + 1) * 128], tp[:, 512:640])

            # ---------------- Phase C/D: attention + Wo ----------------
            o_ts = {}

            def emit_d_group(tt, nch):
                # one quarter of out[tt*128:(tt+1)*128, :] — PE filler
                # injected between attention kt-iterations (its attnT
                # inputs were finalized a block earlier)
                if nch == 0:
                    o_t = outp.tile([128, HID], f32, tag="out")
                    o_ts[tt] = o_t
                o_t = o_ts[tt]
                w_ps = psS.tile([128, 512], f32, tag="psS")
                for db in range(4):
                    nc.tensor.matmul(
                        w_ps[:],
                        lhsT=attnT[:, db, tt * 128:(tt + 1) * 128],
                        rhs=wo_sb[:, db, nch * 512:(nch + 1) * 512],
                        start=(db == 0), stop=(db == 3))
                nc.vector.tensor_copy(
                    o_t[:, nch * 512:(nch + 1) * 512], w_ps[:])
                if nch == 3:
                    nc.sync.dma_start(out=out[tt * 128:(tt + 1) * 128, :],
                                      in_=o_t[:])

            # per (kv, qc) block: 4 q-heads of one kv group over one
            # q-chunk. Emission is software-pipelined: scores(kt) are
            # emitted BEFORE PV(kt-1) so the exp -> scores -> exp critical
            # path never queues behind PV matmuls in the strict-FIFO PE
            # queue. D groups of the previous q-chunk are drip-fed between
            # iterations.
            for qc in range(4):
                for kv in range(2):
                    qr = kv * 64
                    if qc > 0:
                        base = 4 * (qc - 1) + kv * 2
                        dq = [(base + i // 4, i % 4) for i in range(8)]
                    else:
                        dq = []
                    o_ps = []
                    for _i in range(4):
                        acc = psO.tile([65, 512], f32, tag="acc")
                        o_ps.append(acc)
                    plist = [None, None]
                    for kt in range(NT + 1):
                        pnew = [None, None]
                        for pair in range(2):
                            if kt < NT:
                                kblk = KT[kv * 64:(kv + 1) * 64,
                                          kt * 128:(kt + 1) * 128]
                                sp = psS.tile([128, 1024], f32, tag="psS")
                                for j in range(2):
                                    mt = pair * 2 + j
                                    nc.tensor.matmul(
                                        sp[:, j * 512:(j + 1) * 512],
                                        lhsT=kblk,
                                        rhs=QT[qr:qr + 64, mt,
                                               qc * 512:(qc + 1) * 512],
                                        start=True, stop=True)
                                p = pbf.tile([128, 1024], bf16, tag="p")
                                nc.scalar.activation(p[:], sp[:], AF.Exp,
                                                     scale=0.125)
                                pnew[pair] = p
                            if kt > 0:
                                for j in range(2):
                                    nc.tensor.matmul(
                                        o_ps[pair * 2 + j][:],
                                        lhsT=V[:, kt - 1, kv, :],
                                        rhs=plist[pair][:,
                                                        j * 512:(j + 1) * 512],
                                        start=(kt == 1), stop=(kt == NT))
                        plist = pnew
                        if dq and kt >= 2 and kt % 2 == 0:
                            emit_d_group(*dq.pop(0))
                    for h4 in range(4):
                        rsum = rbp.tile([1, 512], f32, tag="rsum")
                        nc.vector.tensor_copy(rsum[:], o_ps[h4][64:65, :])
                        recip = rbp.tile([1, 512], f32, tag="recip")
                        nc.vector.reciprocal_approx_fast(recip[:], rsum[:])
                        rb = rbp.tile([64, 512], f32, tag="rb")
                        nc.gpsimd.partition_broadcast(rb[:], recip[:])
                        nc.vector.tensor_tensor(
                            attnT[qr:qr + 64, h4, qc * 512:(qc + 1) * 512],
                            o_ps[h4][0:64, :], rb[:], OP.mult)
            for i in range(16):
                emit_d_group(12 + i // 4, i % 4)

            if dbg:
                for (dtile, stile) in ((d_qt, QT), (d_kt, KT), (d_v, V),
                                       (d_at, attnT)):
                    nc.gpsimd.dma_start(out=dtile, in_=stile[:])

    nc.compile()
    return nc


def _get_nc():
    if "nc" not in _CACHE:
        _CACHE["nc"] = _build()
    return _CACHE["nc"]


def _rope_tables():
    # cos/sin[p, t, i] at position t*128+p, emb = concat(freqs, freqs)
    inv = 1.0 / (10000.0 ** (np.arange(0, 32, dtype=np.float64) / 32.0))
    pos = np.arange(S, dtype=np.float64)
    fr = np.outer(pos, inv)                       # [S, 32]
    emb = np.concatenate([fr, fr], axis=1)        # [S, 64]
    cos = np.cos(emb).astype(np.float32).reshape(NT, 128, D).transpose(1, 0, 2)
    sin = np.sin(emb).astype(np.float32).reshape(NT, 128, D).transpose(1, 0, 2)
    return np.ascontiguousarray(cos), np.ascontiguousarray(sin)


def _shard(inputs):
    import ml_dtypes
    bf = ml_dtypes.bfloat16
    hs = np.asarray(inputs["hidden_states"], np.float32)
    Wq = np.asarray(inputs["Wq"], np.float32)
    Wk = np.asarray(inputs["Wk"], np.float32)
    Wv = np.asarray(inputs["Wv"], np.float32)
    Wo = np.asarray(inputs["Wo"], np.float32)
    cos, sin = _rope_tables()
    xts = [np.ascontiguousarray(hs[b].T).astype(bf) for b in range(2)]
    in_maps = []
    for i in range(8):
        b, g = divmod(i, 4)
        # wqk columns: [half=2, blk=5, d=64]; blk 0-3 = q head h=half*4+blk,
        # blk 4 = k head kh=half. then v (2 heads x 64) appended.
        cols = []
        for half in range(2):
            for blk in range(5):
                if blk < 4:
                    h = half * 4 + blk
                    cols.append(Wq[:, g * 512 + h * 64: g * 512 + (h + 1) * 64])
                else:
                    cols.append(Wk[:, g * 128 + half * 64:
                                   g * 128 + (half + 1) * 64])
        cols.append(Wv[:, g * 128:(g + 1) * 128])
        wqk = np.concatenate(cols, axis=1).astype(bf)
        wo = np.ascontiguousarray(
            Wo[g * 512:(g + 1) * 512, :].reshape(8, 64, HID)[
                [0, 4, 1, 5, 2, 6, 3, 7]].reshape(512, HID)).astype(bf)
        in_maps.append({
            "xt": xts[b],
            "wqk": np.ascontiguousarray(wqk),
            "wo": wo,
            "rc": cos,
            "rsn": sin,
        })
    return in_maps


def run(inputs, trace=False, tmpdir=None):
    """Run on 8 cores; returns (output [2,2048,2048] f32, exec_time_ns)."""
    from concourse.bass_utils import run_bass_kernel_spmd

    nc = _get_nc()
    in_maps = _shard(inputs)
    kwargs = {}
    if trace:
        import sys, types
        from trn_agent_boot.trn_boot import _ntff_profile_via_ctypes
        if "antenv.axon_hooks" not in sys.modules:
            mod = types.ModuleType("antenv.axon_hooks")
            hook = _ntff_profile_via_ctypes("/opt/axon/libaxon_pjrt.so")
            mod.get_axon_ntff_profile_hook = lambda: hook
            sys.modules["antenv.axon_hooks"] = mod
        import concourse.bass_utils as bu
        bu.upload_artifacts = lambda d: f"local://{d}"
        kwargs = {"trace": True, "tmpdir": tmpdir}
    res = run_bass_kernel_spmd(nc, in_maps, core_ids=list(range(8)), **kwargs)
    full = np.zeros((2, S, HID), np.float32)
    for i in range(8):
        b = i // 4
        full[b] += res.results[i]["out"]
    return full, res.exec_time_ns


def kernel(**inputs):
    out, _ = run(inputs)
    return out
